# revision 10
# baseline (speedup 1.0000x reference)
"""DocRED relation-extraction head on 8 Trainium2 NeuronCores.

Data-parallel over the batch axis: core b owns batch b's hidden_states slab
and its entity/pair indices; the classifier weights are replicated.

The model is fully linear after the gather:
    logits[p] = rep[head[p]] @ W1 @ out_w + rep[tail[p]] @ W2 @ out_w
              + (dense_b @ out_w + out_b)
so the two weight matmuls fold into one replicated matrix at load time:
    Wc = dense_w @ out_w          [2H, C]   (~400KB fp16 vs 8MB dense_w)
    cst = dense_b @ out_w + out_b [C]
(weight folding on the host, once, exact in fp32; everything
data-dependent -- the mention gather, mention-sum, entity projection and
pair combination -- runs on device).

Device pipeline per core:
    gather   128 mention rows of hidden_states (indirect DMA, 256KB)
    repT     mention-sum fused with transpose via 8 matmuls vs block-ones
    eL1/eL2  repT-chunks @ Wc-chunks -> [32, 98] logit-space entity reps
    logits[p] = eL1[head[p]] + eL2[tail[p]] + cst via ONE K=65-stacked
             one-hot matmul per 128-pair tile (the [65, P] one-hot stack is
             host-built from the indices: head rows, tail rows, ones row).

Scheduling notes (from trace analysis): every dma_start costs ~0.7us of
serial descriptor-gen on its ring sequencer, and all rings share the same
16 physical DMA queues FIFO, so the tiny `pos` DMA must be generated
before the bulk `wc` stream or the gather is priority-inverted behind
400KB of weights. cst rides first on the sync ring as a cheap delay so
pos's descriptors win the queue race.

Precision: fp16 operand tiles (hidden_states / Wc / one-hot path);
PE accumulates in fp32. End-to-end ~1e-3 scale-relative vs the fp32
reference.
"""

import numpy as np
from contextlib import ExitStack

import concourse.bass as bass
import concourse.bacc as bacc
import concourse.tile as tile
import concourse.mybir as mybir
from concourse.bass_utils import run_bass_kernel_spmd

B, L, H, E, M, P, C = 8, 2048, 1024, 32, 4, 1024, 97
N_CORES = 8
HC = H // 128    # h-dim 128-chunks
PT = P // 128    # pair tiles
CP = C + 1       # class dim padded to 98 (even moving dim)
K = 2 * E + 1    # stacked one-hot contraction: head rows, tail rows, ones row
NWARM = 68       # PE warm-up matmuls (~53ns each cold; covers t~7.4->11us)

f32 = mybir.dt.float32
f16 = mybir.dt.float16
i32 = mybir.dt.int32

_CACHE = {}


def _build():
    nc = bacc.Bacc("TRN2", target_bir_lowering=False, debug=False)

    hs = nc.dram_tensor("hs", [L, H], f16, kind="ExternalInput").ap()
    pos = nc.dram_tensor("pos", [E * M, 1], i32, kind="ExternalInput").ap()
    onesb = nc.dram_tensor("onesb", [E * M, E], f16, kind="ExternalInput").ap()
    # folded weights: 16 chunks of [128, 98]; chunk c = Wc rows 128c..128c+128
    wc = nc.dram_tensor("wc", [128, 2 * HC * CP], f16, kind="ExternalInput").ap()
    # cst = dense_b @ out_w + out_b -> eL-stack row 64 payload
    cst = nc.dram_tensor("cst", [1, CP], f16, kind="ExternalInput").ap()
    # host-built one-hot stack: rows 0-31 head, 32-63 tail, row 64 ones
    oh = nc.dram_tensor("oh", [K, P], f16, kind="ExternalInput").ap()
    # output laid out [128, PT*C]: pair-tile t in columns t*C..(t+1)*C; host
    # reshapes to [P, C]. fp16: logits max ~9, half-ulp 0.0039 abs -> ~4e-4
    # of output scale, halves the store stream.
    out = nc.dram_tensor("out", [128, PT * C], f16, kind="ExternalOutput").ap()

    with tile.TileContext(nc) as tc, ExitStack() as ctx:
        sb = ctx.enter_context(tc.tile_pool(name="sb", bufs=1))
        pspool = ctx.enter_context(tc.tile_pool(name="ps", bufs=8, space="PSUM"))

        sb_eL = sb.tile([K, CP], f16)

        # ---- input DMAs, priority order. pos rides the gpsimd ring itself so
        # the indirect gather's wait is ring-local and its descriptors win the
        # shared-queue race against the bulk wc stream. cst first on sync
        # delays wc desc-gen just enough that pos's descriptors enqueue first.
        sb_pos = sb.tile([E * M, 1], i32)
        nc.gpsimd.dma_start(sb_pos[:], pos[:])
        sb_ones = sb.tile([E * M, E], f16)
        nc.scalar.dma_start(sb_ones[:], onesb[:])
        sb_oh = sb.tile([K, P], f16)
        nc.scalar.dma_start(sb_oh[:], oh[:])

        nc.sync.dma_start(sb_eL[2 * E:2 * E + 1, :], cst[:])
        sb_wc = sb.tile([128, 2 * HC * CP], f16)
        nc.sync.dma_start(sb_wc[:], wc[:])

        # ---- gather the 128 mention rows of hidden_states
        sb_g = sb.tile([E * M, H], f16)
        nc.gpsimd.indirect_dma_start(
            out=sb_g[:],
            out_offset=None,
            in_=hs[:],
            in_offset=bass.IndirectOffsetOnAxis(ap=sb_pos[:, :1], axis=0),
        )

        # ---- PE warm-up: the HAM clock gate holds an idle PE at 1.2 GHz and
        # needs ~3.4us of sustained activity to release to 2.4 GHz. Burn
        # discarded f32 matmuls (2 ISA passes each) on a memset tile.
        wdum = sb.tile([128, E], f32)
        nc.vector.memset(wdum[:], 0.0)
        ps_warm = pspool.tile([E, E], f32, tag="ps")
        for i in range(NWARM):
            nc.tensor.matmul(
                out=ps_warm[:],
                lhsT=wdum[:],
                rhs=wdum[:],
                start=True,
                stop=True,
            )

        # ---- stage A: entity_repT[h, e] = sum_m gathered[4e+m, h]
        # (mention-sum and transpose fused into 8 matmuls vs block-ones);
        # 4 chunks per PSUM bank, one copy per bank.
        sb_repT = sb.tile([128, HC * E], f16)
        for g in range(2):
            pa = pspool.tile([128, 4 * E], f32, tag="ps", name=f"pa{g}")
            for q in range(4):
                hc = g * 4 + q
                nc.tensor.matmul(
                    out=pa[:, q * E:(q + 1) * E],
                    lhsT=sb_g[:, hc * 128:(hc + 1) * 128],
                    rhs=sb_ones[:],
                    start=True,
                    stop=True,
                )
            nc.vector.tensor_copy(
                out=sb_repT[:, g * 4 * E:(g + 1) * 4 * E], in_=pa[:])

        # ---- stage B: eL1 = rep @ Wc1, eL2 = rep @ Wc2  [32, 98] each.
        # hc-outer so each repT chunk is loaded as stationary once.
        ps_eL = [pspool.tile([E, CP], f32, tag="ps", name=f"ps_eL{h}")
                 for h in range(2)]
        for hc in range(HC):
            for half in range(2):
                nc.tensor.matmul(
                    out=ps_eL[half][:],
                    lhsT=sb_repT[:, hc * E:(hc + 1) * E],
                    rhs=sb_wc[:, (half * HC + hc) * CP:(half * HC + hc + 1) * CP],
                    start=(hc == 0),
                    stop=(hc == HC - 1),
                )

        # ---- eL stack [65, 98]: rows 0-31 = eL1, 32-63 = eL2, row 64 = cst
        # (already DMA'd).
        nc.vector.tensor_copy(out=sb_eL[:E, :], in_=ps_eL[0][:])
        nc.vector.tensor_copy(out=sb_eL[E:2 * E, :], in_=ps_eL[1][:])

        # ---- stage D: stacked one-hot pair gather; 4 tiles per PSUM bank.
        sb_out = sb.tile([128, PT * C], f16)
        for g in range(2):
            pl = pspool.tile([128, 4 * CP], f32, tag="ps", name=f"pl{g}")
            for q in range(4):
                pt = g * 4 + q
                nc.tensor.matmul(
                    out=pl[:, q * CP:(q + 1) * CP],
                    lhsT=sb_oh[:, pt * 128:(pt + 1) * 128],
                    rhs=sb_eL[:],
                    start=True,
                    stop=True,
                )
            nc.vector.tensor_copy(
                out=sb_out[:].rearrange("p (t c) -> p t c", c=C)[:, g * 4:(g + 1) * 4, :],
                in_=pl[:].rearrange("p (t c) -> p t c", c=CP)[:, :, :C],
            )
        # store split across both rings; host reshapes to [1024, 97]
        nc.scalar.dma_start(out[:, :PT * C // 2], sb_out[:, :PT * C // 2])
        nc.sync.dma_start(out[:, PT * C // 2:], sb_out[:, PT * C // 2:])

    nc.compile()
    return nc


def get_compiled():
    if "nc" not in _CACHE:
        _CACHE["nc"] = _build()
    return _CACHE["nc"]


def make_in_maps(hidden_states, dense_w, dense_b, out_w, out_b,
                 entity_position_ids, head_tail_idxs):
    hidden_states = np.asarray(hidden_states)
    dense_w = np.asarray(dense_w, dtype=np.float32)
    dense_b = np.asarray(dense_b, dtype=np.float32)
    out_w = np.asarray(out_w, dtype=np.float32)
    out_b = np.asarray(out_b, dtype=np.float32)
    entity_position_ids = np.asarray(entity_position_ids)
    head_tail_idxs = np.asarray(head_tail_idxs)

    # fold the classifier: Wc = dense_w @ out_w, cst = dense_b @ out_w + out_b
    wc_full = dense_w @ out_w                        # [2H, C] fp32
    cst = dense_b @ out_w + out_b                    # [C]
    wcp = np.zeros((2 * H, CP), np.float32)
    wcp[:, :C] = wc_full
    # device layout: 16 chunks [128, 98] side by side
    wc_dev = np.ascontiguousarray(
        wcp.reshape(2 * HC, 128, CP).transpose(1, 0, 2).reshape(128, 2 * HC * CP)
    ).astype(np.float16)
    cst_dev = np.zeros((1, CP), np.float16)
    cst_dev[0, :C] = cst.astype(np.float16)

    ids = np.arange(E, dtype=np.int32)
    in_maps = []
    for b in range(B):
        ht = head_tail_idxs[b]  # [P, 2] int32
        oh = np.empty((K, P), np.float16)
        oh[:E, :] = (ids[:, None] == ht[None, :, 0])
        oh[E:2 * E, :] = (ids[:, None] == ht[None, :, 1])
        oh[2 * E, :] = 1.0
        in_maps.append({
            "hs": np.ascontiguousarray(hidden_states[b], dtype=np.float16),
            "pos": np.ascontiguousarray(
                entity_position_ids[b].reshape(E * M, 1).astype(np.int32)),
            "onesb": np.repeat(np.eye(E, dtype=np.float16), M, axis=0),
            "wc": wc_dev,
            "cst": cst_dev,
            "oh": oh,
        })
    return in_maps


def kernel(hidden_states, dense_w, dense_b, out_w, out_b,
           entity_position_ids, head_tail_idxs, _trace=False, _trace_kwargs=None):
    nc = get_compiled()
    in_maps = make_in_maps(hidden_states, dense_w, dense_b, out_w, out_b,
                           entity_position_ids, head_tail_idxs)
    res = run_bass_kernel_spmd(
        nc, in_maps, core_ids=list(range(N_CORES)),
        trace=_trace, **(_trace_kwargs or {}),
    )
    outp = np.concatenate(
        [res.results[i]["out"].astype(np.float32).reshape(128, PT, C)
         .transpose(1, 0, 2).reshape(P, C) for i in range(N_CORES)], axis=0)
    if _trace:
        return outp, res
    return outp
